# revision 10
# baseline (speedup 1.0000x reference)
"""Trainium2 Bass kernel for CapsNet dynamic routing (nn_Capsule_13692355740297).

Math (per batch element b, I=4096 input caps, Din=128, N=10 out caps, D=16):
    u_hat[i,(n,d)] = u[i,:] @ W[:,(n,d)]                 # never materialized
    iter1: c uniform 1/10 -> s1 = 0.1 * (sum_i u_i) @ W
    iter k: b[i,n] = u_i . v_n,  v_n = W_n @ o_n         # PE, contract Din
            c = softmax_n(b)                             # [i-part, n-free]
            R[d,n] = sum_i c[i,n] u[i,d]                 # PE, contract i
            s[n,:] = colsum_d(R[:,n] * W[:,(n,:)])       # vector mult + ones-matmul
            o = squash(s)

Key implementation choices vs a naive port:
  - All of U is fp16 (host-cast, round-to-nearest): rel err ~5e-4, DMA halves.
  - U is loaded in BOTH layouts (natural [i-part, d] and transposed [d-part, i]),
    both host-prepared so no on-chip transposes and all DMAs are contiguous.
  - i-grouping: i = p*32 + j (p = partition, j = tile), so the natural-layout DMA
    is 8KB-contiguous per partition per batch.
  - Per-batch software pipeline: slot b runs iter2(b) interleaved with iter3(b-1)
    so PE never waits on a softmax/squash chain; DMA (1MB per transfer) overlaps.
  - PE warmup matmuls at t=0 flip the HAM clock gate to 2.4GHz before routing.
Sharding: data-parallel over batch, 8 batch elements per core, no collectives.
"""

import numpy as np

B, I_FULL, DIN = 64, 4096, 128
NCAP, DCAP = 10, 16
KND = NCAP * DCAP  # 160
NCORES = 8
BC = B // NCORES  # 8 batch elements per core
NT = I_FULL // 128  # 32 i-tiles per batch
IL = I_FULL
EPS = 1e-7


def build_nc(bc=BC, nt=NT):
    import concourse.bacc as bacc
    import concourse.mybir as mybir
    from concourse.tile import TileContext

    fp32 = mybir.dt.float32
    fp16 = mybir.dt.float16
    AX = mybir.AxisListType
    ALU = mybir.AluOpType
    ACTF = mybir.ActivationFunctionType

    il = nt * 128

    nc = bacc.Bacc(trn_type="TRN2")
    un_h = nc.dram_tensor("un", [bc, 128, il], fp16, kind="ExternalInput")
    ut_h = nc.dram_tensor("ut", [128, bc * il], fp16, kind="ExternalInput")
    w32_h = nc.dram_tensor("w32", [DIN, KND], fp32, kind="ExternalInput")
    w16_h = nc.dram_tensor("w16", [DIN, KND], fp16, kind="ExternalInput")
    wth_h = nc.dram_tensor("wth", [128, DIN], fp16, kind="ExternalInput")
    wtl_h = nc.dram_tensor("wtl", [32, DIN], fp16, kind="ExternalInput")
    mh_h = nc.dram_tensor("mh", [128, NCAP], fp16, kind="ExternalInput")
    ml_h = nc.dram_tensor("ml", [32, NCAP], fp16, kind="ExternalInput")
    idf_h = nc.dram_tensor("idf", [128, 128], fp16, kind="ExternalInput")
    ones_h = nc.dram_tensor("ones", [128, 1], fp16, kind="ExternalInput")
    out_h = nc.dram_tensor("out", [bc, KND], fp32, kind="ExternalOutput")

    with TileContext(nc) as tc:
        with (
            tc.tile_pool(name="big", bufs=1) as big,
            tc.tile_pool(name="sb2", bufs=2) as sb2,
            tc.tile_pool(name="sb3", bufs=3) as sb3,
            tc.tile_pool(name="psB", bufs=2, space="PSUM") as psB,
            tc.tile_pool(name="psR", bufs=2, space="PSUM") as psR,
            tc.tile_pool(name="psS", bufs=2, space="PSUM") as psS,
            tc.tile_pool(name="psT", bufs=2, space="PSUM") as psT,
        ):
            # ---------- persistent SBUF ----------
            U = big.tile([128, bc * il], fp16, name="U_sb")    # [p, (b, j, d)] i=p*32+j
            UT = big.tile([128, bc * il], fp16, name="UT_sb")  # [d, (b, j, p)]
            w32 = big.tile([128, KND], fp32, name="w32_sb")
            w16 = big.tile([128, KND], fp16, name="w16_sb")
            wth = big.tile([128, DIN], fp16, name="wth_sb")
            wtl = big.tile([32, DIN], fp16, name="wtl_sb")
            mh = big.tile([128, NCAP], fp16, name="mh_sb")
            ml = big.tile([32, NCAP], fp16, name="ml_sb")
            idf = big.tile([128, 128], fp16, name="idf_sb")
            ones = big.tile([128, 1], fp16, name="ones_sb")

            wmsrc = big.tile([128, 128], fp16, name="wmsrc_sb")

            Wv = w32[:, :].rearrange("p (n d) -> p n d", n=NCAP)

            # ---------- PE warmup (keep HAM busy so routing runs at 2.4GHz) ----
            # Reads a memset tile so it has no DMA dependency and starts at t=0.
            nc.vector.memset(wmsrc[:, :], 0.0)
            wm = psB.tile([128, nt * NCAP], fp32, name="warm", tag="btp")
            for k in range(64):
                nc.tensor.matmul(wm[:, :16], wmsrc[:, :], wmsrc[:, :16])

            # ---------- loads ----------
            nc.sync.dma_start(out=w32[:, :], in_=w32_h.ap())
            nc.sync.dma_start(out=w16[:, :], in_=w16_h.ap())
            nc.sync.dma_start(out=wth[:, :], in_=wth_h.ap())
            nc.sync.dma_start(out=wtl[:, :], in_=wtl_h.ap())
            nc.sync.dma_start(out=mh[:, :], in_=mh_h.ap())
            nc.sync.dma_start(out=ml[:, :], in_=ml_h.ap())
            nc.sync.dma_start(out=idf[:, :], in_=idf_h.ap())
            nc.sync.dma_start(out=ones[:, :], in_=ones_h.ap())
            for b in range(bc):
                nc.sync.dma_start(
                    out=UT[:, b * il : (b + 1) * il],
                    in_=ut_h.ap()[:, b * il : (b + 1) * il],
                )
                nc.sync.dma_start(out=U[:, b * il : (b + 1) * il], in_=un_h.ap()[b])

            def ut_tile(b, j):
                return UT[:, b * il + 128 * j : b * il + 128 * (j + 1)]

            def u_tile(b, j):
                return U[:, b * il + 128 * j : b * il + 128 * (j + 1)]

            # ---------- helpers ----------
            def squash(s_row, out_ap, key):
                """out = squash(s_row [1,KND] f32); out_ap may be fp16 or f32."""
                sq = sb2.tile([1, KND], fp32, name=f"sq{key}", tag="sq")
                qq = sb2.tile([1, NCAP], fp32, name=f"qq{key}", tag="qq")
                rt = sb2.tile([1, NCAP], fp32, name=f"rt{key}", tag="rt")
                den = sb2.tile([1, NCAP], fp32, name=f"den{key}", tag="den")
                rden = sb2.tile([1, NCAP], fp32, name=f"rden{key}", tag="rden")
                coef = sb2.tile([1, NCAP], fp32, name=f"coef{key}", tag="coef")
                nc.gpsimd.tensor_tensor(
                    out=sq[:, :], in0=s_row[:, :], in1=s_row[:, :], op=ALU.mult
                )
                nc.vector.reduce_sum(
                    out=qq[:, :],
                    in_=sq[:, :].rearrange("a (n d) -> a n d", n=NCAP),
                    axis=AX.X,
                )
                # EPS=1e-7 is negligible vs q (and den>=1 regardless): drop it
                nc.scalar.activation(rt[:, :], qq[:, :], ACTF.Sqrt, bias=0.0)
                nc.gpsimd.tensor_scalar_add(den[:, :], qq[:, :], 1.0)
                nc.vector.reciprocal(out=rden[:, :], in_=den[:, :])
                nc.gpsimd.tensor_tensor(
                    out=coef[:, :], in0=rt[:, :], in1=rden[:, :], op=ALU.mult
                )
                nc.vector.tensor_tensor(
                    out=out_ap.rearrange("a (n d) -> a n d", n=NCAP),
                    in0=s_row[:, :].rearrange("a (n d) -> a n d", n=NCAP),
                    in1=coef[:, :].unsqueeze(2).broadcast_to([1, NCAP, DCAP]),
                    op=ALU.mult,
                )

            def make_v(o16, key, tag):
                """V[d,n] = W_n @ o_n from o16 [1,KND] fp16 -> V [128,NCAP] fp16."""
                oth_p = psT.tile([128, 1], fp16, name=f"othp{key}", tag="tp")
                otl_p = psT.tile([32, 1], fp16, name=f"otlp{key}", tag="tp")
                nc.tensor.transpose(oth_p[:, :], o16[:, 0:128], idf[:1, :1])
                nc.tensor.transpose(otl_p[:, :], o16[:, 128:KND], idf[:1, :1])
                oth = sb2.tile([128, 1], fp16, name=f"oth{key}", tag="oth")
                otl = sb2.tile([32, 1], fp16, name=f"otl{key}", tag="otl")
                nc.scalar.copy(out=oth[:, :], in_=oth_p[:, :])
                nc.scalar.copy(out=otl[:, :], in_=otl_p[:, :])
                oeh = sb2.tile([128, NCAP], fp16, name=f"oeh{key}", tag="oeh")
                oel = sb2.tile([32, NCAP], fp16, name=f"oel{key}", tag="oel")
                nc.gpsimd.tensor_tensor(
                    out=oeh[:, :],
                    in0=oth[:, :].broadcast_to([128, NCAP]),
                    in1=mh[:, :],
                    op=ALU.mult,
                )
                nc.gpsimd.tensor_tensor(
                    out=oel[:, :],
                    in0=otl[:, :].broadcast_to([32, NCAP]),
                    in1=ml[:, :],
                    op=ALU.mult,
                )
                vp = psT.tile([128, NCAP], fp32, name=f"vp{key}", tag="tp")
                nc.tensor.matmul(vp[:, :], wth[:, :], oeh[:, :], start=True, stop=False)
                nc.tensor.matmul(vp[:, :], wtl[:, :], oel[:, :], start=False, stop=True)
                V = sb2.tile([128, NCAP], fp16, name=f"V{key}", tag=tag)
                nc.scalar.copy(out=V[:, :], in_=vp[:, :])
                return V

            def phase1(b):
                """r0 -> s1 -> o1 -> V_it2[b] (uniform-c iteration, batch-local)."""
                r0 = sb2.tile([128, 1], fp32, name=f"r0_{b}", tag="r0")
                nc.vector.reduce_sum(
                    out=r0[:, :], in_=UT[:, b * il : (b + 1) * il], axis=AX.X
                )
                r016 = sb2.tile([128, 1], fp16, name=f"r016_{b}", tag="r016")
                nc.gpsimd.tensor_scalar_add(r016[:, :], r0[:, :], 0.0)
                s1p = psS.tile([1, KND], fp32, name=f"s1p{b}", tag="sp")
                nc.tensor.matmul(s1p[:, :], r016[:, :], w16[:, :])
                s_row = sb3.tile([1, KND], fp32, name=f"s1row{b}", tag="srow")
                nc.scalar.mul(out=s_row[:, :], in_=s1p[:, :], mul=0.1)
                o16 = sb2.tile([1, KND], fp16, name=f"o1_{b}", tag="o16")
                squash(s_row, o16[:, :], f"1_{b}")
                return make_v(o16, f"1_{b}", "V2")

            def b_pass(b, V, it):
                """c = softmax_n(U_b @ V) -> cc [128,(j,n)] fp16."""
                btp = psB.tile([128, nt * NCAP], fp32, name=f"btp{it}_{b}", tag="btp")
                for j in range(nt):
                    nc.tensor.matmul(
                        btp[:, NCAP * j : NCAP * (j + 1)], ut_tile(b, j), V[:, :]
                    )
                eb = sb3.tile([128, nt * NCAP], fp32, name=f"eb{it}_{b}", tag="eb")
                nc.scalar.activation(eb[:, :], btp[:, :], ACTF.Exp)
                ebv = eb[:, :].rearrange("p (j n) -> p j n", j=nt)
                Z = sb2.tile([128, nt], fp32, name=f"Z{it}_{b}", tag="Z")
                nc.vector.reduce_sum(out=Z[:, :], in_=ebv, axis=AX.X)
                rZ = sb2.tile([128, nt], fp32, name=f"rZ{it}_{b}", tag="rZ")
                nc.vector.reciprocal(out=rZ[:, :], in_=Z[:, :])
                cc = sb3.tile([128, nt * NCAP], fp16, name=f"cc{it}_{b}", tag="cc")
                nc.vector.tensor_tensor(
                    out=cc[:, :].rearrange("p (j n) -> p j n", j=nt),
                    in0=ebv,
                    in1=rZ[:, :].unsqueeze(2).broadcast_to([128, nt, NCAP]),
                    op=ALU.mult,
                )
                return cc

            def r_pass(b, cc, it):
                """R = U_b^T cc; s = colsum(R*W) -> s_row [1,KND] f32 SBUF."""
                Rp = psR.tile([128, NCAP], fp32, name=f"Rp{it}_{b}", tag="Rp")
                for j in range(nt):
                    nc.tensor.matmul(
                        Rp[:, :],
                        u_tile(b, j),
                        cc[:, NCAP * j : NCAP * (j + 1)],
                        start=(j == 0),
                        stop=(j == nt - 1),
                    )
                prod = sb2.tile([128, KND], fp16, name=f"prod{it}_{b}", tag="prod")
                nc.vector.tensor_tensor(
                    out=prod[:, :].rearrange("p (n d) -> p n d", n=NCAP),
                    in0=Rp[:, :].unsqueeze(2).broadcast_to([128, NCAP, DCAP]),
                    in1=Wv,
                    op=ALU.mult,
                )
                sp = psS.tile([1, KND], fp32, name=f"sp{it}_{b}", tag="sp")
                nc.tensor.matmul(sp[:, :], ones[:, :], prod[:, :])
                s_row = sb3.tile([1, KND], fp32, name=f"srow{it}_{b}", tag="srow")
                nc.scalar.copy(out=s_row[:, :], in_=sp[:, :])
                return s_row

            # ---------- software-pipelined routing ----------
            # slot b: b2(b), b3(b-1), R2(b), R3(b-1); V for slot b+2 prepped here.
            V2 = [None] * bc  # V for iter2 (from o1)
            V3 = [None] * bc  # V for iter3 (from o2)
            cc2 = [None] * bc
            cc3 = [None] * bc

            V2[0] = phase1(0)

            for b in range(bc + 1):
                if b < bc:
                    cc2[b] = b_pass(b, V2[b], 2)
                if b == 0 and bc > 1:
                    # emitted after b2(0) so its PE ops (gated on batch 1's DMA)
                    # don't head-of-line-block the first b-pass
                    V2[1] = phase1(1)
                if b >= 1:
                    cc3[b - 1] = b_pass(b - 1, V3[b - 1], 3)
                if b < bc:
                    s2 = r_pass(b, cc2[b], 2)
                    o16 = sb2.tile([1, KND], fp16, name=f"o2_{b}", tag="o16")
                    squash(s2, o16[:, :], f"2_{b}")
                    V3[b] = make_v(o16, f"2_{b}", "V3")
                if b >= 1:
                    s3 = r_pass(b - 1, cc3[b - 1], 3)
                    o3row = sb3.tile([1, KND], fp32, name=f"o3_{b - 1}", tag="o3row")
                    squash(s3, o3row[:, :], f"3_{b - 1}")
                    nc.sync.dma_start(out=out_h.ap()[b - 1], in_=o3row[:, :])
                if b + 2 < bc:
                    V2[b + 2] = phase1(b + 2)

    nc.compile()
    return nc


def make_const_inputs():
    ident = np.eye(128, dtype=np.float16)
    mask = np.zeros((KND, NCAP), dtype=np.float16)
    for k in range(KND):
        mask[k, k // DCAP] = 1.0
    return {
        "idf": ident,
        "mh": mask[:128],
        "ml": mask[128:],
        "ones": np.ones((128, 1), dtype=np.float16),
    }


def make_w_inputs(W):
    W = np.asarray(W, dtype=np.float32)
    WT16 = np.ascontiguousarray(W.T).astype(np.float16)  # [160, 128]
    return {
        "w32": W,
        "w16": W.astype(np.float16),
        "wth": WT16[:128],
        "wtl": WT16[128:],
    }


def make_u_inputs(u_vecs):
    """Per-core natural + transposed fp16 layouts of u.

    un[c][b, p, m*128+d] = u[c*BC+b, p*32+m, d]        (contiguous view)
    ut[c][d, b*4096 + j*128 + p] = u[c*BC+b, p*32+j, d]
    """
    u16 = np.asarray(u_vecs, dtype=np.float32).astype(np.float16)
    uns, uts = [], []
    for c in range(NCORES):
        blk = u16[c * BC : (c + 1) * BC]  # [BC, 4096, 128]
        uns.append(np.ascontiguousarray(blk.reshape(BC, 128, IL)))
        ut = np.empty((128, BC, NT, 128), dtype=np.float16)
        for b in range(BC):
            t = np.ascontiguousarray(blk[b].T)  # [128 d, 4096 i] i=(p,m)
            ut[:, b] = t.reshape(128, 128, NT).swapaxes(1, 2)  # [d, j, p]
        uts.append(ut.reshape(128, BC * IL))
    return uns, uts


_CACHE = {}


def make_in_maps(u_vecs, W):
    consts = make_const_inputs()
    wis = make_w_inputs(W)
    uns, uts = make_u_inputs(u_vecs)
    in_maps = []
    for c in range(NCORES):
        m = {"un": uns[c], "ut": uts[c]}
        m.update(consts)
        m.update(wis)
        in_maps.append(m)
    return in_maps


def kernel(u_vecs, W):
    from concourse import bass_utils

    if "nc" not in _CACHE:
        _CACHE["nc"] = build_nc()
    nc = _CACHE["nc"]

    in_maps = make_in_maps(u_vecs, W)
    res = bass_utils.run_bass_kernel_spmd(nc, in_maps, core_ids=list(range(NCORES)))
    outs = [r["out"] for r in res.results]
    return np.concatenate(outs, axis=0).reshape(B, NCAP, DCAP).astype(np.float32)
